# revision 30
# baseline (speedup 1.0000x reference)
"""EdgeOnlyConv GNN message-passing kernel for Trainium2 (8 NeuronCores).

out[e] = concat(x[src[e]], x[dest[e]], edge_attr[e]) @ W.T + b
       = Ys[src[e]] + Yd[dest[e]] + edge_attr[e] @ We.T        (Ys folds bias)

Gather-free edge-parallel design (v2).  dma_gather descriptor generation on
the Q7 SWDGE path costs ~8.6 ns/descriptor (2 of 8 GpSimd cores), which
capped the previous kernel at ~2.2 ms for 250k descriptors/core.  This
version never generates per-edge descriptors:

  Host: per core and per endpoint, sort edges by node id; greedily cut the
    sorted stream into <=512-edge chunks whose node-id span fits a 256-row
    slab; emit per-chunk slab bases, per-edge slab-local indices (int8,
    biased by -128), endpoint-permuted edge_attr, and column->edge maps.
  Device phase A: node tables Ys = x@Wsrc.T + b, Yd = x@Wdst.T (fp16,
    node-major) built on PE in one pass over xT, stored to DRAM.
  Device per chunk (both passes interleaved):
    - dynamic-offset DMA stages the 256-row slab [128p, 2slot, 128f]
    - gpsimd.partition_broadcast replicates the int8 local-idx row
    - DVE is_equal vs per-partition constants -> two one-hot f16 tiles
    - PE: psum[f,e] = slab0.T@E0 + slab1.T@E1 (+ We.T@edge_attr on pass A)
    - ACT copies psum -> fp16 staging, feature-major DMA store
  Host: un-permute the two partial outputs (f32) and add.
"""

import sys
import numpy as np

if "/opt/trn_rl_repo" not in sys.path:
    sys.path.insert(0, "/opt/trn_rl_repo")

P = 128
CH = 512          # edge columns per chunk (PSUM bank = 512 f32)
SLAB = 256        # node rows staged per chunk (2 matmul slots)
N_CORES = 8
N_NODES = 50000
N_IN_NODE = 128
N_IN_EDGE = 64
N_OUT = 128
N_EDGES = 1000000
E_CORE = N_EDGES // N_CORES            # 125000
NODES_PAD = (N_NODES + P - 1) // P * P  # 50176
A_TILES = NODES_PAD // P               # 392


def build_program(n_ch, nodes_pad=NODES_PAD, n_cores=N_CORES):
    """n_ch: chunks per pass (same for all cores; host pads to this)."""
    import concourse.mybir as mybir
    import concourse.tile as tile
    from concourse import bacc
    from concourse import bass as cbass

    f32 = mybir.dt.float32
    f16 = mybir.dt.float16
    i8 = mybir.dt.int8
    i32 = mybir.dt.int32
    EQ = mybir.AluOpType.is_equal
    ds = cbass.ds

    e_dev = n_ch * CH
    a_tiles = nodes_pad // P

    nc = bacc.Bacc("TRN2", target_bir_lowering=False, debug=False,
                   num_devices=n_cores)

    xT_d = nc.dram_tensor("xT", [P, nodes_pad], f16, kind="ExternalInput").ap()
    wsT_d = nc.dram_tensor("wsT", [P, P], f16, kind="ExternalInput").ap()
    wdT_d = nc.dram_tensor("wdT", [P, P], f16, kind="ExternalInput").ap()
    weT_d = nc.dram_tensor("weT", [N_IN_EDGE, P], f16, kind="ExternalInput").ap()
    bias_d = nc.dram_tensor("bias", [P, P], f32, kind="ExternalInput").ap()
    pidx_d = nc.dram_tensor("pidx", [P, CH // 2], f16, kind="ExternalInput").ap()
    basAa_d = nc.dram_tensor("basAa", [1, n_ch], i32, kind="ExternalInput").ap()
    basAb_d = nc.dram_tensor("basAb", [1, n_ch], i32, kind="ExternalInput").ap()
    basBa_d = nc.dram_tensor("basBa", [1, n_ch], i32, kind="ExternalInput").ap()
    basBb_d = nc.dram_tensor("basBb", [1, n_ch], i32, kind="ExternalInput").ap()
    liA_d = nc.dram_tensor("liA", [1, e_dev], f16, kind="ExternalInput").ap()
    liB_d = nc.dram_tensor("liB", [1, e_dev], f16, kind="ExternalInput").ap()
    eatA_d = nc.dram_tensor("eatA", [N_IN_EDGE, e_dev], f16, kind="ExternalInput").ap()
    outA_d = nc.dram_tensor("outA", [P, e_dev], f16, kind="ExternalOutput").ap()
    outB_d = nc.dram_tensor("outB", [P, e_dev], f16, kind="ExternalOutput").ap()
    ys_d = nc.dram_tensor("ys", [nodes_pad, P], f16, kind="Internal").ap()
    yd_d = nc.dram_tensor("yd", [nodes_pad, P], f16, kind="Internal").ap()

    GRP = 16   # node tiles per phase-A group
    LGRP = 8   # chunks per li-row load
    OGRP = 4   # chunks per output staging group

    with tile.TileContext(nc) as tc:
        with tc.tile_pool(name="static", bufs=1) as spool:
            wsT_sb = spool.tile([P, P], f16)
            nc.sync.dma_start(wsT_sb[:], wsT_d[:, :])
            wdT_sb = spool.tile([P, P], f16)
            nc.sync.dma_start(wdT_sb[:], wdT_d[:, :])
            weT_sb = spool.tile([N_IN_EDGE, P], f16)
            nc.sync.dma_start(weT_sb[:], weT_d[:, :])
            bias_sb = spool.tile([P, P], f32)
            nc.sync.dma_start(bias_sb[:], bias_d[:, :])
            pidx_sb = spool.tile([P, CH // 2], f16)
            nc.sync.dma_start(pidx_sb[:], pidx_d[:, :])
            basAa_sb = spool.tile([1, n_ch], i32)
            nc.sync.dma_start(basAa_sb[:], basAa_d[:, :])
            basAb_sb = spool.tile([1, n_ch], i32)
            nc.scalar.dma_start(basAb_sb[:], basAb_d[:, :])
            basBa_sb = spool.tile([1, n_ch], i32)
            nc.sync.dma_start(basBa_sb[:], basBa_d[:, :])
            basBb_sb = spool.tile([1, n_ch], i32)
            nc.scalar.dma_start(basBb_sb[:], basBb_d[:, :])

            # ---- Phase A: Ys = x@Wsrc.T + b, Yd = x@Wdst.T (node-major) ----
            with tc.tile_pool(name="pa", bufs=2) as papool, \
                 tc.tile_pool(name="paps", bufs=4, space="PSUM") as paps:
                for g0 in range(0, a_tiles, GRP):
                    gn = min(GRP, a_tiles - g0)
                    xt = papool.tile([P, GRP * P], f16, tag="xt")
                    nc.sync.dma_start(xt[:, :gn * P],
                                      xT_d[:, g0 * P:(g0 + gn) * P])
                    ysb = papool.tile([P, GRP, P], f16, tag="ysb")
                    ydb = papool.tile([P, GRP, P], f16, tag="ydb")
                    for t in range(gn):
                        ps = paps.tile([P, 2 * P], f32, tag="ps")
                        nc.tensor.matmul(ps[:, 0:P],
                                         lhsT=xt[:, t * P:(t + 1) * P],
                                         rhs=wsT_sb[:], start=True, stop=True)
                        nc.tensor.matmul(ps[:, P:2 * P],
                                         lhsT=xt[:, t * P:(t + 1) * P],
                                         rhs=wdT_sb[:], start=True, stop=True)
                        nc.vector.tensor_add(ysb[:, t, :], ps[:, 0:P], bias_sb[:])
                        nc.scalar.copy(ydb[:, t, :], ps[:, P:2 * P])
                    ys_rows = ys_d[g0 * P:(g0 + gn) * P, :].rearrange(
                        "(t p) f -> p t f", p=P)
                    yd_rows = yd_d[g0 * P:(g0 + gn) * P, :].rearrange(
                        "(t p) f -> p t f", p=P)
                    nc.sync.dma_start(ys_rows[:, :, :], ysb[:, :gn, :])
                    nc.gpsimd.dma_start(yd_rows[:, :, :], ydb[:, :gn, :])

            tc.strict_bb_all_engine_barrier()

            # ---- Passes A (src) and B (dst), interleaved chunk loop ----
            # each 512-col chunk = two independent 256-edge halves, each with
            # its own 128-row slab (contraction 128 -> 1 matmul cycle/edge)
            with tc.tile_pool(name="pb", bufs=3) as bpool, \
                 tc.tile_pool(name="bps", bufs=2, space="PSUM") as bps:
                passes = [
                    ("A", (basAa_sb, basAb_sb), liA_d, ys_d, outA_d, True),
                    ("B", (basBa_sb, basBb_sb), liB_d, yd_d, outB_d, False),
                ]
                HC = CH // 2
                li_rows = {}
                eat_rows = {}
                outst = {}
                half_engs = [nc.sync, nc.scalar]
                # per engine: 2 cycling banks x (2 passes x RB) registers
                RB = 4
                sregs = [[[e.alloc_register(f"sb{h}_{i}_{j}")
                           for j in range(2 * RB)] for h in range(2)]
                         for i, e in enumerate(half_engs)]
                ncopy = [0]
                for k in range(n_ch):
                    for (tagp, bas_sbs, li_d, tab_d, out_d, has_z) in passes:
                        pi = 0 if tagp == "A" else 1
                        if k % LGRP == 0:
                            lw = min(LGRP, n_ch - k) * CH
                            lr = bpool.tile([P, LGRP * CH], f16,
                                            tag=f"li{tagp}")
                            nc.gpsimd.dma_start(
                                lr[:, :lw],
                                li_d[0:1, k * CH:k * CH + lw].to_broadcast(
                                    [P, lw]))
                            li_rows[tagp] = lr
                            if has_z:
                                er = bpool.tile([N_IN_EDGE, LGRP * CH], f16,
                                                tag="eat")
                                nc.gpsimd.dma_start(
                                    er[:, :lw],
                                    eatA_d[:, k * CH:k * CH + lw])
                                eat_rows[tagp] = er
                        if k % RB == 0:
                            rn = min(RB, n_ch - k)
                            for hi in (0, 1):
                                bank = sregs[hi][(k // RB) % 2]
                                half_engs[hi].reg_load(
                                    bank[pi * RB:pi * RB + rn],
                                    bas_sbs[hi][0:1, k:k + rn])
                        slab = bpool.tile([P, 2, P], f16, tag=f"slab{tagp}")
                        for hi in (0, 1):
                            eng = half_engs[hi]
                            r = sregs[hi][(k // RB) % 2][pi * RB + k % RB]
                            base = eng.snap(r, min_val=0,
                                            max_val=nodes_pad - P)
                            eng.dma_start(slab[:, hi, :],
                                          tab_d[ds(base, P), :])
                        libc = li_rows[tagp][
                            :, (k % LGRP) * CH:(k % LGRP + 1) * CH]
                        eT0 = bpool.tile([P, HC], f16, tag=f"eT0{tagp}")
                        nc.vector.tensor_tensor(
                            eT0[:, :], libc[:, 0:HC], pidx_sb[:, :], op=EQ)
                        eT1 = bpool.tile([P, HC], f16, tag=f"eT1{tagp}")
                        nc.vector.tensor_tensor(
                            eT1[:, :], libc[:, HC:CH], pidx_sb[:, :], op=EQ)
                        # two PSUM banks per chunk: half h in bank h at
                        # col h*CH; start/stop ranges match exactly per bank
                        po = bps.tile([P, 2 * CH], f32, tag=f"po{tagp}")
                        if has_z:
                            ec = eat_rows[tagp]
                            e0 = (k % LGRP) * CH
                            nc.tensor.matmul(
                                po[:, 0:HC], lhsT=weT_sb[:],
                                rhs=ec[:, e0:e0 + HC],
                                start=True, stop=False)
                            nc.tensor.matmul(
                                po[:, 0:HC], lhsT=slab[:, 0, :],
                                rhs=eT0[:, :], start=False, stop=True)
                            nc.tensor.matmul(
                                po[:, CH:CH + HC], lhsT=weT_sb[:],
                                rhs=ec[:, e0 + HC:e0 + CH],
                                start=True, stop=False)
                            nc.tensor.matmul(
                                po[:, CH:CH + HC], lhsT=slab[:, 1, :],
                                rhs=eT1[:, :], start=False, stop=True)
                        else:
                            nc.tensor.matmul(po[:, 0:HC], lhsT=slab[:, 0, :],
                                             rhs=eT0[:, :], start=True,
                                             stop=True)
                            nc.tensor.matmul(po[:, CH:CH + HC],
                                             lhsT=slab[:, 1, :],
                                             rhs=eT1[:, :], start=True,
                                             stop=True)
                        if k % OGRP == 0:
                            ot = bpool.tile([P, OGRP * CH], f16,
                                            tag=f"outst{tagp}",
                                            name=f"outst{tagp}_{k}")
                            outst[tagp] = ot
                        # drain psum: 1 of 3 on DVE, rest on ACT
                        po_v = po.rearrange("p (b c) -> p b c", b=2)[:, :, 0:HC]
                        od = outst[tagp][:, (k % OGRP) * CH:
                                         (k % OGRP + 1) * CH].rearrange(
                                             "p (b c) -> p b c", b=2)
                        ncopy[0] += 1
                        if ncopy[0] % 3 == 0:
                            nc.vector.tensor_copy(od, po_v)
                        else:
                            nc.scalar.copy(od, po_v)
                        if k % OGRP == OGRP - 1 or k == n_ch - 1:
                            k0 = k - (k % OGRP)
                            nc.sync.dma_start(
                                out_d[:, k0 * CH:(k + 1) * CH],
                                outst[tagp][:, :(k % OGRP + 1) * CH])

    nc.compile()
    return nc


def _plan_pass(key_sorted):
    """Greedy half-chunking of a sorted node-id stream.

    Returns list of (start, count, base): count <= CH//2 edges starting at
    `start` whose ids fit in [base, base+128)."""
    n = len(key_sorted)
    hc = CH // 2
    halves = []
    i = 0
    while i < n:
        base = int(key_sorted[i])
        base = min(base, NODES_PAD - P)
        j = min(i + hc, n)
        j = i + int(np.searchsorted(key_sorted[i:j], base + P, side="left"))
        halves.append((i, j - i, base))
        i = j
    return halves


def prep_core(src, dst, edge_attr_core):
    """Per-core host prep. Returns dict of device arrays + colmaps."""
    plans = {}
    for tagp, key in (("A", src), ("B", dst)):
        perm = np.argsort(key, kind="stable")
        ks = key[perm].astype(np.int64)
        chunks = _plan_pass(ks)
        plans[tagp] = (perm, ks, chunks)
    return plans


def pack_core(plans, edge_attr_core, n_ch):
    e_dev = n_ch * CH
    dev = {}
    colmaps = {}
    hc = CH // 2
    for tagp, (perm, ks, halves) in plans.items():
        li = np.zeros(e_dev, dtype=np.float16)
        basesa = np.zeros(n_ch, dtype=np.int32)
        basesb = np.zeros(n_ch, dtype=np.int32)
        colmap = np.full(e_dev, -1, dtype=np.int64)
        for h, (s, cnt, base) in enumerate(halves):
            c, hi = divmod(h, 2)
            (basesa if hi == 0 else basesb)[c] = base
            o = c * CH + hi * hc
            li[o:o + cnt] = (ks[s:s + cnt] - base).astype(np.float16)
            colmap[o:o + cnt] = perm[s:s + cnt]
        dev[f"li{tagp}"] = li[None, :]
        dev[f"bas{tagp}a"] = basesa[None, :]
        dev[f"bas{tagp}b"] = basesb[None, :]
        colmaps[tagp] = colmap
        if tagp == "A":
            eat = np.zeros((N_IN_EDGE, e_dev), dtype=np.float16)
            valid = colmap >= 0
            eat[:, valid] = edge_attr_core[colmap[valid]].astype(np.float16).T
            dev["eatA"] = eat
    return dev, colmaps


def prep_inputs(x, edge_index, edge_attr, W, b):
    """Host-side prep: shard + sort + pack. Returns (in_maps, colmaps, n_ch)."""
    x = np.asarray(x, dtype=np.float32)
    edge_index = np.asarray(edge_index)
    edge_attr = np.asarray(edge_attr, dtype=np.float32)
    W = np.asarray(W, dtype=np.float32)
    b = np.asarray(b, dtype=np.float32)

    xT = np.zeros((P, NODES_PAD), dtype=np.float16)
    xT[:, :N_NODES] = x.astype(np.float16).T
    wsT = np.ascontiguousarray(W[:, :P].T).astype(np.float16)
    wdT = np.ascontiguousarray(W[:, P:2 * P].T).astype(np.float16)
    weT = np.ascontiguousarray(W[:, 2 * P:].T).astype(np.float16)
    bias_rep = np.ascontiguousarray(
        np.tile(b[None, :].astype(np.float32), (P, 1)))
    pidx = np.tile(np.arange(P, dtype=np.float16)[:, None], (1, CH // 2))

    src = np.ascontiguousarray(edge_index[0]).astype(np.int64)
    dst = np.ascontiguousarray(edge_index[1]).astype(np.int64)

    core_plans = []
    n_ch = 0
    for c in range(N_CORES):
        lo, hi = c * E_CORE, (c + 1) * E_CORE
        plans = prep_core(src[lo:hi], dst[lo:hi], None)
        for tagp in ("A", "B"):
            n_ch = max(n_ch, (len(plans[tagp][2]) + 1) // 2)
        core_plans.append(plans)

    in_maps = []
    all_colmaps = []
    for c in range(N_CORES):
        lo, hi = c * E_CORE, (c + 1) * E_CORE
        dev, colmaps = pack_core(core_plans[c], edge_attr[lo:hi], n_ch)
        dev.update({
            "xT": xT, "wsT": wsT, "wdT": wdT, "weT": weT,
            "bias": bias_rep, "pidx": pidx,
        })
        in_maps.append(dev)
        all_colmaps.append(colmaps)
    return in_maps, all_colmaps, n_ch


_NC_CACHE = {}


def _get_program(n_ch):
    if n_ch not in _NC_CACHE:
        _NC_CACHE[n_ch] = build_program(n_ch)
    return _NC_CACHE[n_ch]


def run_on_hw(in_maps, nc=None, trace=False, n_cores=N_CORES):
    from concourse import bass_utils
    if nc is None:
        raise ValueError("pass nc")
    kw = {}
    if trace:
        _install_profile_hook(bass_utils)
        kw["trace"] = True
    res = bass_utils.run_bass_kernel_spmd(
        nc, in_maps, core_ids=list(range(n_cores)), **kw)
    return res


def _install_profile_hook(bass_utils):
    """Inject the NTFF profile hook missing from this image's antenv."""
    import types
    if "antenv.axon_hooks" in sys.modules:
        return
    try:
        from trn_agent_boot.trn_boot import _ntff_profile_via_ctypes
        hook = _ntff_profile_via_ctypes("/opt/axon/libaxon_pjrt.so")
    except Exception:
        hook = None
    mod = types.ModuleType("antenv.axon_hooks")
    mod.get_axon_ntff_profile_hook = lambda: hook
    mod.set_axon_ntff_profile_hook = lambda h: None
    sys.modules["antenv.axon_hooks"] = mod
    bass_utils.upload_artifacts = lambda tmpdir: f"file://{tmpdir}"


def combine_outputs(res, all_colmaps):
    out = np.zeros((N_EDGES, N_OUT), dtype=np.float32)
    for c in range(N_CORES):
        lo = c * E_CORE
        cmA = all_colmaps[c]["A"]
        cmB = all_colmaps[c]["B"]
        outA = np.asarray(res.results[c]["outA"])  # [128, e_dev] f16
        outB = np.asarray(res.results[c]["outB"])
        vA = cmA >= 0
        vB = cmB >= 0
        out[lo + cmA[vA]] = outA[:, vA].T.astype(np.float32)
        out[lo + cmB[vB]] += outB[:, vB].T.astype(np.float32)
    return out


def kernel(x, edge_index, edge_attr, W, b):
    in_maps, all_colmaps, n_ch = prep_inputs(x, edge_index, edge_attr, W, b)
    nc = _get_program(n_ch)
    res = run_on_hw(in_maps, nc=nc)
    return combine_outputs(res, all_colmaps)
